# revision 15
# baseline (speedup 1.0000x reference)
"""Trainium2 Bass kernel for nn_AttributeBiasLoss.

Reference computation:
    per_node = mean(sigmoid(predictions), axis=1)            # [B]
    for each attribute a: group per_node by attr_vals[:, a] (V=16 values)
    means[a, v] = mean of per_node over group (a, v)
    loss = sum over attrs of pairwise squared diffs of present group means
           / number of comparisons

Kernel strategy (data-parallel over 8 cores, batch-sharded):
  One-hot + TensorEngine reduction.  Per 196-column chunk group:
    - ACT: sigmoid(pred);  DVE: t8 = sum_d sigmoid (tensor_reduce, bf16 out)
    - DVE: H[p, c, v, a] = (x[p,c,a] == v) as bf16 via 16 tensor_scalar
      is_equal passes (4x perf mode: 2-byte dtype, packed inner dim, SBUF)
    - PE : per group of W=4 columns, one matmul with block weights
      lhsT = interleaved [t8_c | 1] pairs (128 x 2W) against the contiguous
      one-hot run rhs (128 x W*128, bf16 max f1 within one PSUM bank):
      PSUM[2W, W*128] accumulates; diagonal blocks (row 2w, col block w)
      hold 8*segment-sums (row 2w) and counts (row 2w+1); cross blocks are
      garbage ignored at extraction.  Grouping amortizes the per-matmul
      weight reload (424 ns/column ungrouped -> ~220 ns effective/group).
  Pads use x=16 -> one-hot all zero -> excluded automatically.
  AllReduce the raw [1,4096] PSUM block; per-core epilogue extracts the
  diagonals and computes the loss via the centered-variance identity.
"""

import sys

sys.path.insert(0, "/opt/trn_rl_repo")

from contextlib import ExitStack

import numpy as np

import concourse.bacc as bacc
import concourse.bass as bass
import concourse.mybir as mybir
import concourse.tile as tile
from concourse import bass_utils
from concourse._compat import with_exitstack

F32 = mybir.dt.float32
BF16 = mybir.dt.bfloat16
I32 = mybir.dt.int32
AF = mybir.ActivationFunctionType
OP = mybir.AluOpType

# Problem constants (hardcoded per harness contract).
B, D, A, V = 2_000_000, 8, 8, 16
NCORES = 8
ROWS_PER_CORE = B // NCORES  # 250_000

PAD_ATTR = 16  # out-of-range bucket: one-hot all-zero, excluded from stats

SUB = 10
CSUB = 196
CP = SUB * CSUB  # 1960
ROWS_PAD = 128 * CP  # 250_880
NSTAT = 2 * A * V  # 256


@with_exitstack
def emit_kernel(
    ctx: ExitStack,
    tc: tile.TileContext,
    pred_d,  # DRAM [ROWS_PAD, D] f32
    attr_d,  # DRAM [ROWS_PAD, A] bf16 (row-major, host-cast)
    loss_d,  # DRAM [1, 1] f32
    n_cores=NCORES,
):
    nc = tc.nc
    cp, sub, csub = CP, SUB, CSUB

    io = ctx.enter_context(tc.tile_pool(name="io", bufs=3))
    hp = ctx.enter_context(tc.tile_pool(name="h", bufs=2))
    wp = ctx.enter_context(tc.tile_pool(name="w", bufs=1))
    smallp = ctx.enter_context(tc.tile_pool(name="small", bufs=1))
    psump = ctx.enter_context(tc.tile_pool(name="ps", bufs=1, space="PSUM"))
    dramp = ctx.enter_context(tc.tile_pool(name="dram", bufs=1, space="DRAM"))

    # Stationary weights, interleaved pairs: [p, c, 0] = t8, [p, c, 1] = 1.
    t8ones = wp.tile([128, cp, 2], BF16, name="t8ones")
    nc.vector.memset(t8ones[:, :, 1], 1.0)

    # Warm up the collective engine early (channel setup dominates the first
    # collective); no data dependencies so it overlaps the main compute.
    NFLATW = 2 * 4 * 4 * A * V
    warm_in = dramp.tile([1, NFLATW], F32, name="warm_in")
    warm_out = dramp.tile([1, NFLATW], F32, name="warm_out")
    warm_s = smallp.tile([1, NFLATW], F32, name="warm_s")
    nc.vector.memset(warm_s[:], 0.0)
    nc.sync.dma_start(warm_in[:], warm_s[:])
    nc.gpsimd.collective_compute(
        "AllReduce",
        OP.add,
        replica_groups=[list(range(n_cores))],
        ins=[warm_in.opt()],
        outs=[warm_out.opt()],
    )

    pred_v = pred_d.rearrange("(p s c) d -> s p (c d)", p=128, s=sub)
    attr_v = attr_d.rearrange("(p s c) a -> s p (c a)", p=128, s=sub)

    # PSUM accumulator for W-grouped matmuls: rows (2w, 2w+1) x cols (w, av).
    W = 4  # out [2W, W*128] fp32 = 2KB/partition: fits one PSUM bank
    stats_ps = psump.tile([2 * W, W * A * V], F32)
    ngrp_full, tail = divmod(csub, W)

    for s in range(sub):
        x_t = io.tile([128, csub * A], BF16, tag="attr")
        nc.sync.dma_start(x_t[:], attr_v[s])
        pred_t = io.tile([128, csub * D], F32, tag="pred")
        nc.sync.dma_start(pred_t[:], pred_v[s])
        x_ca = x_t.rearrange("p (c a) -> p c a", a=A)

        sig_t = io.tile([128, csub * D], F32, tag="sig")
        nc.scalar.activation(sig_t[:], pred_t[:], AF.Sigmoid)

        # One-hot H[p, c, v, a] over this chunk group (bf16, 4x perf mode:
        # both operands 2-byte, inner dim a contiguous).  Emitted before the
        # row-sum so the DVE starts as soon as the attr DMA lands.
        h_t = hp.tile([128, csub, V, A], BF16, tag="h")
        for v in range(V):
            nc.vector.tensor_scalar(
                out=h_t[:, :, v, :],
                in0=x_ca,
                scalar1=float(v),
                scalar2=None,
                op0=OP.is_equal,
            )

        with nc.allow_low_precision(reason="bf16 t8 adds ~1e-6 rel to means"):
            nc.vector.tensor_reduce(
                t8ones[:, s * csub : (s + 1) * csub, 0],
                sig_t.rearrange("p (c d) -> p c d", d=D),
                op=OP.add,
                axis=mybir.AxisListType.X,
            )

        # PE: per group of W columns, block weights [.., (w, [t8|1])] against
        # rhs [.., (w, av)]: out row 2w ( = t8_cw * H_cw ) and 2w+1 (counts)
        # land in column block w; cross terms are ignored at extraction.
        h_f = h_t.rearrange("p c v a -> p (c v a)")
        ngrp = ngrp_full + (1 if tail else 0)
        for g in range(ngrp):
            c0 = g * W
            w = W if g < ngrp_full else tail
            c0_abs = s * csub + c0
            nc.tensor.matmul(
                stats_ps[: 2 * w, : w * A * V],
                lhsT=t8ones[:, c0_abs : c0_abs + w, :].rearrange(
                    "p w two -> p (w two)"
                ),
                rhs=h_f[:, c0 * A * V : (c0 + w) * A * V],
                start=(s == 0 and g == 0),
                stop=(s == sub - 1 and g == ngrp - 1),
            )

    # Extract the diagonal blocks: S = sum_w psum[2w, w*128+av],
    # n = sum_w psum[2w+1, w*128+av].  Cross blocks are garbage.  Engines
    # cannot address partitions at an offset, so flatten the [2W, W*128]
    # block onto partition 0 via an SBUF->SBUF DMA first.
    sb_stats = smallp.tile([2 * W, W * A * V], F32, name="sb_stats")
    nc.vector.tensor_copy(sb_stats[:], stats_ps[:])

    # AllReduce the raw [2W, W*128] block (as [1, 4096]); extract after.
    NFLAT = 2 * W * W * A * V
    cc_in = dramp.tile([1, NFLAT], F32, name="cc_in")
    cc_out = dramp.tile([1, NFLAT], F32, name="cc_out")
    nc.sync.dma_start(cc_in[:], sb_stats[:])
    nc.gpsimd.collective_compute(
        "AllReduce",
        OP.add,
        replica_groups=[list(range(n_cores))],
        ins=[cc_in.opt()],
        outs=[cc_out.opt()],
    )
    gf = smallp.tile([1, NFLAT], F32, name="gf")
    nc.sync.dma_start(gf[:], cc_out[:])
    g = smallp.tile([1, NSTAT], F32, name="g")
    row = W * A * V
    for r in range(2):
        dst = g[:, r * A * V : (r + 1) * A * V]
        nc.vector.tensor_copy(dst, gf[:, r * row : r * row + A * V])
        for w in range(1, W):
            off = (2 * w + r) * row + w * A * V
            nc.vector.tensor_tensor(
                out=dst, in0=dst, in1=gf[:, off : off + A * V], op=OP.add
            )

    # ---------------- epilogue (tiny, partition 0, redundant per core) -------
    ep = ctx.enter_context(tc.tile_pool(name="ep", bufs=1))

    S8 = g[:, 0 : A * V].rearrange("p (v a) -> p a v", v=V)  # 8 * segment sums
    n_t = g[:, A * V : NSTAT].rearrange("p (v a) -> p a v", v=V)  # counts

    # m = S8 / (8 * max(n, 1))
    nmax = ep.tile([1, A * V], F32, name="nmax")
    nc.vector.tensor_scalar(
        out=nmax[:].rearrange("p (a v) -> p a v", a=A),
        in0=n_t,
        scalar1=1.0,
        scalar2=None,
        op0=OP.max,
    )
    rn = ep.tile([1, A * V], F32, name="rn")
    nc.vector.reciprocal(rn[:], nmax[:])
    m = ep.tile([1, A * V], F32, name="m").rearrange("p (a v) -> p a v", a=A)
    nc.vector.scalar_tensor_tensor(
        out=m,
        in0=S8,
        scalar=1.0 / 8.0,
        in1=rn.rearrange("p (a v) -> p a v", a=A),
        op0=OP.mult,
        op1=OP.mult,
    )

    # present mask & per-attr stats
    p_t = ep.tile([1, A * V], F32, name="p").rearrange("p (a v) -> p a v", a=A)
    nc.vector.tensor_scalar(out=p_t, in0=n_t, scalar1=0.5, scalar2=None, op0=OP.is_ge)
    k_t = ep.tile([1, A], F32, name="k")
    nc.vector.tensor_reduce(k_t[:], p_t, op=OP.add, axis=mybir.AxisListType.X)

    mp = ep.tile([1, A * V], F32, name="mp").rearrange("p (a v) -> p a v", a=A)
    nc.vector.tensor_tensor(out=mp, in0=m, in1=p_t, op=OP.mult)
    ms = ep.tile([1, A], F32, name="ms")
    nc.vector.tensor_reduce(ms[:], mp, op=OP.add, axis=mybir.AxisListType.X)

    kmax = ep.tile([1, A], F32, name="kmax")
    nc.vector.tensor_scalar(
        out=kmax[:], in0=k_t[:], scalar1=1.0, scalar2=None, op0=OP.max
    )
    rk = ep.tile([1, A], F32, name="rk")
    nc.vector.reciprocal(rk[:], kmax[:])
    mu = ep.tile([1, A], F32, name="mu")
    nc.vector.tensor_tensor(out=mu[:], in0=ms[:], in1=rk[:], op=OP.mult)

    # d = (m - mu) * present ; q = sum_v d^2 ; contrib = k * q
    dtile = ep.tile([1, A * V], F32, name="d").rearrange("p (a v) -> p a v", a=A)
    nc.vector.scalar_tensor_tensor(
        out=dtile,
        in0=mu[:].broadcast_to([1, A, V]),
        scalar=-1.0,
        in1=m,
        op0=OP.mult,
        op1=OP.add,
    )
    nc.vector.tensor_tensor(out=dtile, in0=dtile, in1=p_t, op=OP.mult)
    d2 = ep.tile([1, A * V], F32, name="d2").rearrange("p (a v) -> p a v", a=A)
    nc.vector.tensor_tensor(out=d2, in0=dtile, in1=dtile, op=OP.mult)
    q_t = ep.tile([1, A], F32, name="q")
    nc.vector.tensor_reduce(q_t[:], d2, op=OP.add, axis=mybir.AxisListType.X)

    contrib = ep.tile([1, A], F32, name="contrib")
    nc.vector.tensor_tensor(out=contrib[:], in0=k_t[:], in1=q_t[:], op=OP.mult)
    tot = ep.tile([1, 1], F32, name="tot")
    nc.vector.tensor_reduce(tot[:], contrib[:], op=OP.add, axis=mybir.AxisListType.X)

    # ncomp = sum_a k(k-1)/2
    kk = ep.tile([1, A], F32, name="kk")
    nc.vector.scalar_tensor_tensor(
        out=kk[:], in0=k_t[:], scalar=-1.0, in1=k_t[:], op0=OP.add, op1=OP.mult
    )
    ncomp = ep.tile([1, 1], F32, name="ncomp")
    nc.vector.tensor_reduce(ncomp[:], kk[:], op=OP.add, axis=mybir.AxisListType.X)
    nc.vector.tensor_scalar(
        out=ncomp[:], in0=ncomp[:], scalar1=0.5, scalar2=None, op0=OP.mult
    )

    # loss = (ncomp > 0) * tot / max(ncomp, 0.5)
    ncm = ep.tile([1, 1], F32, name="ncm")
    nc.vector.tensor_scalar(
        out=ncm[:], in0=ncomp[:], scalar1=0.5, scalar2=None, op0=OP.max
    )
    rnc = ep.tile([1, 1], F32, name="rnc")
    nc.vector.reciprocal(rnc[:], ncm[:])
    mask = ep.tile([1, 1], F32, name="mask")
    nc.vector.tensor_scalar(
        out=mask[:], in0=ncomp[:], scalar1=0.25, scalar2=None, op0=OP.is_ge
    )
    res = ep.tile([1, 1], F32, name="res")
    nc.vector.tensor_tensor(out=res[:], in0=tot[:], in1=rnc[:], op=OP.mult)
    nc.vector.tensor_tensor(out=res[:], in0=res[:], in1=mask[:], op=OP.mult)

    nc.sync.dma_start(loss_d[:], res[:])


def build(n_cores=NCORES):
    nc = bacc.Bacc(
        "TRN2", target_bir_lowering=False, debug=False, num_devices=n_cores
    )
    pred_d = nc.dram_tensor("pred", [ROWS_PAD, D], F32, kind="ExternalInput").ap()
    attr_d = nc.dram_tensor("attr", [ROWS_PAD, A], BF16, kind="ExternalInput").ap()
    loss_d = nc.dram_tensor("loss", [1, 1], F32, kind="ExternalOutput").ap()
    with tile.TileContext(nc) as tc:
        emit_kernel(tc, pred_d, attr_d, loss_d, n_cores=n_cores)
    nc.compile()
    return nc


def shard_inputs(predictions, attr_vals, n_cores=NCORES, rows_pad=ROWS_PAD):
    rows = predictions.shape[0] // n_cores
    in_maps = []
    for c in range(n_cores):
        p = predictions[c * rows : (c + 1) * rows]
        a = attr_vals[c * rows : (c + 1) * rows]
        pad = rows_pad - rows
        if pad:
            p = np.concatenate([p, np.zeros((pad, D), np.float32)], axis=0)
            a = np.concatenate([a, np.full((pad, A), PAD_ATTR, np.int32)], axis=0)
        a16 = a.astype(np.float32).astype(_BF16_NP)
        in_maps.append(
            {
                "pred": np.ascontiguousarray(p),
                "attr": np.ascontiguousarray(a16),
            }
        )
    return in_maps


try:
    import ml_dtypes

    _BF16_NP = ml_dtypes.bfloat16
except Exception:  # pragma: no cover
    import jax.numpy as jnp

    _BF16_NP = jnp.bfloat16

_NC_CACHE = {}


def kernel(predictions: np.ndarray, attr_vals: np.ndarray) -> np.ndarray:
    predictions = np.asarray(predictions, np.float32)
    attr_vals = np.asarray(attr_vals, np.int32)
    if "nc" not in _NC_CACHE:
        _NC_CACHE["nc"] = build()
    nc = _NC_CACHE["nc"]
    in_maps = shard_inputs(predictions, attr_vals)
    res = bass_utils.run_bass_kernel_spmd(nc, in_maps, list(range(NCORES)))
    return np.float32(res.results[0]["loss"][0, 0])


# revision 16
# speedup vs baseline: 1.1784x; 1.1784x over previous
"""Trainium2 Bass kernel for nn_AttributeBiasLoss.

Reference computation:
    per_node = mean(sigmoid(predictions), axis=1)            # [B]
    for each attribute a: group per_node by attr_vals[:, a] (V=16 values)
    means[a, v] = mean of per_node over group (a, v)
    loss = sum over attrs of pairwise squared diffs of present group means
           / number of comparisons

Kernel strategy (data-parallel over 8 cores, batch-sharded):
  One-hot + TensorEngine reduction.  Per 196-column chunk group:
    - ACT: sigmoid(pred);  DVE: t8 = sum_d sigmoid (tensor_reduce, bf16 out)
    - DVE: H[p, c, v, a] = (x[p,c,a] == v) as bf16 via 16 tensor_scalar
      is_equal passes (4x perf mode: 2-byte dtype, packed inner dim, SBUF)
    - PE : per group of W=4 columns, one matmul with block weights
      lhsT = interleaved [t8_c | 1] pairs (128 x 2W) against the contiguous
      one-hot run rhs (128 x W*128, bf16 max f1 within one PSUM bank):
      PSUM[2W, W*128] accumulates; diagonal blocks (row 2w, col block w)
      hold 8*segment-sums (row 2w) and counts (row 2w+1); cross blocks are
      garbage ignored at extraction.  Grouping amortizes the per-matmul
      weight reload (424 ns/column ungrouped -> ~220 ns effective/group).
  Pads use x=16 -> one-hot all zero -> excluded automatically.
  AllReduce the raw [1,4096] PSUM block; per-core epilogue extracts the
  diagonals and computes the loss via the centered-variance identity.
"""

import sys

sys.path.insert(0, "/opt/trn_rl_repo")

from contextlib import ExitStack

import numpy as np

import concourse.bacc as bacc
import concourse.bass as bass
import concourse.mybir as mybir
import concourse.tile as tile
from concourse import bass_utils
from concourse._compat import with_exitstack

F32 = mybir.dt.float32
BF16 = mybir.dt.bfloat16
I32 = mybir.dt.int32
AF = mybir.ActivationFunctionType
OP = mybir.AluOpType

# Problem constants (hardcoded per harness contract).
B, D, A, V = 2_000_000, 8, 8, 16
NCORES = 8
ROWS_PER_CORE = B // NCORES  # 250_000

PAD_ATTR = 16  # out-of-range bucket: one-hot all-zero, excluded from stats

SUB = 10
CSUB = 196
CP = SUB * CSUB  # 1960
ROWS_PAD = 128 * CP  # 250_880
NSTAT = 2 * A * V  # 256


@with_exitstack
def emit_kernel(
    ctx: ExitStack,
    tc: tile.TileContext,
    pred_d,  # DRAM [ROWS_PAD, D] f32
    attr_d,  # DRAM [ROWS_PAD, A] bf16 (row-major, host-cast)
    loss_d,  # DRAM [1, 1] f32
    n_cores=NCORES,
):
    nc = tc.nc
    cp, sub, csub = CP, SUB, CSUB

    io = ctx.enter_context(tc.tile_pool(name="io", bufs=3))
    hp = ctx.enter_context(tc.tile_pool(name="h", bufs=2))
    wp = ctx.enter_context(tc.tile_pool(name="w", bufs=1))
    smallp = ctx.enter_context(tc.tile_pool(name="small", bufs=1))
    psump = ctx.enter_context(tc.tile_pool(name="ps", bufs=1, space="PSUM"))
    dramp = ctx.enter_context(tc.tile_pool(name="dram", bufs=1, space="DRAM"))

    # Stationary weights, interleaved pairs: [p, c, 0] = t8, [p, c, 1] = 1.
    t8ones = wp.tile([128, cp, 2], BF16, name="t8ones")

    # Warm up the collective engine early (channel setup dominates the first
    # collective); no data dependencies so it overlaps the main compute.
    NFLATW = 64
    warm_in = dramp.tile([1, NFLATW], F32, name="warm_in")
    warm_out = dramp.tile([1, NFLATW], F32, name="warm_out")
    warm_s = smallp.tile([1, NFLATW], F32, name="warm_s")
    nc.vector.memset(warm_s[:], 0.0)
    nc.sync.dma_start(warm_in[:], warm_s[:])
    nc.gpsimd.collective_compute(
        "AllReduce",
        OP.add,
        replica_groups=[list(range(n_cores))],
        ins=[warm_in.opt()],
        outs=[warm_out.opt()],
    )

    pred_v = pred_d.rearrange("(p s c) d -> s p (c d)", p=128, s=sub)
    attr_v = attr_d.rearrange("(p s c) a -> s p (c a)", p=128, s=sub)

    # PSUM accumulator for W-grouped matmuls: rows (2w, 2w+1) x cols (w, av).
    W = 4  # out [2W, W*128] fp32 = 2KB/partition: fits one PSUM bank
    stats_ps = psump.tile([2 * W, W * A * V], F32)
    ngrp_full, tail = divmod(csub, W)

    for s in range(sub):
        x_t = io.tile([128, csub * A], BF16, tag="attr")
        nc.sync.dma_start(x_t[:], attr_v[s])
        pred_t = io.tile([128, csub * D], F32, tag="pred")
        nc.sync.dma_start(pred_t[:], pred_v[s])
        x_ca = x_t.rearrange("p (c a) -> p c a", a=A)

        sig_t = io.tile([128, csub * D], F32, tag="sig")
        nc.scalar.activation(sig_t[:], pred_t[:], AF.Sigmoid)

        # One-hot H[p, c, v, a] over this chunk group (bf16, 4x perf mode:
        # both operands 2-byte, inner dim a contiguous).  Emitted before the
        # row-sum so the DVE starts as soon as the attr DMA lands.
        h_t = hp.tile([128, csub, V, A], BF16, tag="h")
        for v in range(V):
            nc.vector.tensor_scalar(
                out=h_t[:, :, v, :],
                in0=x_ca,
                scalar1=float(v),
                scalar2=None,
                op0=OP.is_equal,
            )

        if s == 0:
            nc.vector.memset(t8ones[:, :, 1], 1.0)
        with nc.allow_low_precision(reason="bf16 t8 adds ~1e-6 rel to means"):
            nc.vector.tensor_reduce(
                t8ones[:, s * csub : (s + 1) * csub, 0],
                sig_t.rearrange("p (c d) -> p c d", d=D),
                op=OP.add,
                axis=mybir.AxisListType.X,
            )

        # PE: per group of W columns, block weights [.., (w, [t8|1])] against
        # rhs [.., (w, av)]: out row 2w ( = t8_cw * H_cw ) and 2w+1 (counts)
        # land in column block w; cross terms are ignored at extraction.
        h_f = h_t.rearrange("p c v a -> p (c v a)")
        ngrp = ngrp_full + (1 if tail else 0)
        for g in range(ngrp):
            c0 = g * W
            w = W if g < ngrp_full else tail
            c0_abs = s * csub + c0
            nc.tensor.matmul(
                stats_ps[: 2 * w, : w * A * V],
                lhsT=t8ones[:, c0_abs : c0_abs + w, :].rearrange(
                    "p w two -> p (w two)"
                ),
                rhs=h_f[:, c0 * A * V : (c0 + w) * A * V],
                start=(s == 0 and g == 0),
                stop=(s == sub - 1 and g == ngrp - 1),
            )

    # Extract the diagonal blocks: S = sum_w psum[2w, w*128+av],
    # n = sum_w psum[2w+1, w*128+av].  Cross blocks are garbage.  Engines
    # cannot address partitions at an offset, so flatten the [2W, W*128]
    # block onto partition 0 via an SBUF->SBUF DMA first.
    sb_stats = smallp.tile([2 * W, W * A * V], F32, name="sb_stats")
    nc.vector.tensor_copy(sb_stats[:], stats_ps[:])

    # AllReduce the raw [2W, W*128] block (as [1, 4096]); extract after.
    NFLAT = 2 * W * W * A * V
    cc_in = dramp.tile([1, NFLAT], F32, name="cc_in")
    cc_out = dramp.tile([1, NFLAT], F32, name="cc_out")
    nc.sync.dma_start(cc_in[:], sb_stats[:])
    nc.gpsimd.collective_compute(
        "AllReduce",
        OP.add,
        replica_groups=[list(range(n_cores))],
        ins=[cc_in.opt()],
        outs=[cc_out.opt()],
    )
    gf = smallp.tile([1, NFLAT], F32, name="gf")
    nc.sync.dma_start(gf[:], cc_out[:])
    g = smallp.tile([1, NSTAT], F32, name="g")
    row = W * A * V
    for r in range(2):
        dst = g[:, r * A * V : (r + 1) * A * V]
        nc.vector.tensor_copy(dst, gf[:, r * row : r * row + A * V])
        for w in range(1, W):
            off = (2 * w + r) * row + w * A * V
            nc.vector.tensor_tensor(
                out=dst, in0=dst, in1=gf[:, off : off + A * V], op=OP.add
            )

    # ---------------- epilogue (tiny, partition 0, redundant per core) -------
    ep = ctx.enter_context(tc.tile_pool(name="ep", bufs=1))

    S8 = g[:, 0 : A * V].rearrange("p (v a) -> p a v", v=V)  # 8 * segment sums
    n_t = g[:, A * V : NSTAT].rearrange("p (v a) -> p a v", v=V)  # counts

    # m = S8 / (8 * max(n, 1))
    nmax = ep.tile([1, A * V], F32, name="nmax")
    nc.vector.tensor_scalar(
        out=nmax[:].rearrange("p (a v) -> p a v", a=A),
        in0=n_t,
        scalar1=1.0,
        scalar2=None,
        op0=OP.max,
    )
    rn = ep.tile([1, A * V], F32, name="rn")
    nc.vector.reciprocal(rn[:], nmax[:])
    m = ep.tile([1, A * V], F32, name="m").rearrange("p (a v) -> p a v", a=A)
    nc.vector.scalar_tensor_tensor(
        out=m,
        in0=S8,
        scalar=1.0 / 8.0,
        in1=rn.rearrange("p (a v) -> p a v", a=A),
        op0=OP.mult,
        op1=OP.mult,
    )

    # present mask & per-attr stats
    p_t = ep.tile([1, A * V], F32, name="p").rearrange("p (a v) -> p a v", a=A)
    nc.vector.tensor_scalar(out=p_t, in0=n_t, scalar1=0.5, scalar2=None, op0=OP.is_ge)
    k_t = ep.tile([1, A], F32, name="k")
    nc.vector.tensor_reduce(k_t[:], p_t, op=OP.add, axis=mybir.AxisListType.X)

    mp = ep.tile([1, A * V], F32, name="mp").rearrange("p (a v) -> p a v", a=A)
    nc.vector.tensor_tensor(out=mp, in0=m, in1=p_t, op=OP.mult)
    ms = ep.tile([1, A], F32, name="ms")
    nc.vector.tensor_reduce(ms[:], mp, op=OP.add, axis=mybir.AxisListType.X)

    kmax = ep.tile([1, A], F32, name="kmax")
    nc.vector.tensor_scalar(
        out=kmax[:], in0=k_t[:], scalar1=1.0, scalar2=None, op0=OP.max
    )
    rk = ep.tile([1, A], F32, name="rk")
    nc.vector.reciprocal(rk[:], kmax[:])
    mu = ep.tile([1, A], F32, name="mu")
    nc.vector.tensor_tensor(out=mu[:], in0=ms[:], in1=rk[:], op=OP.mult)

    # d = (m - mu) * present ; q = sum_v d^2 ; contrib = k * q
    dtile = ep.tile([1, A * V], F32, name="d").rearrange("p (a v) -> p a v", a=A)
    nc.vector.scalar_tensor_tensor(
        out=dtile,
        in0=mu[:].broadcast_to([1, A, V]),
        scalar=-1.0,
        in1=m,
        op0=OP.mult,
        op1=OP.add,
    )
    nc.vector.tensor_tensor(out=dtile, in0=dtile, in1=p_t, op=OP.mult)
    d2 = ep.tile([1, A * V], F32, name="d2").rearrange("p (a v) -> p a v", a=A)
    nc.vector.tensor_tensor(out=d2, in0=dtile, in1=dtile, op=OP.mult)
    q_t = ep.tile([1, A], F32, name="q")
    nc.vector.tensor_reduce(q_t[:], d2, op=OP.add, axis=mybir.AxisListType.X)

    contrib = ep.tile([1, A], F32, name="contrib")
    nc.vector.tensor_tensor(out=contrib[:], in0=k_t[:], in1=q_t[:], op=OP.mult)
    tot = ep.tile([1, 1], F32, name="tot")
    nc.vector.tensor_reduce(tot[:], contrib[:], op=OP.add, axis=mybir.AxisListType.X)

    # ncomp = sum_a k(k-1)/2
    kk = ep.tile([1, A], F32, name="kk")
    nc.vector.scalar_tensor_tensor(
        out=kk[:], in0=k_t[:], scalar=-1.0, in1=k_t[:], op0=OP.add, op1=OP.mult
    )
    ncomp = ep.tile([1, 1], F32, name="ncomp")
    nc.vector.tensor_reduce(ncomp[:], kk[:], op=OP.add, axis=mybir.AxisListType.X)
    nc.vector.tensor_scalar(
        out=ncomp[:], in0=ncomp[:], scalar1=0.5, scalar2=None, op0=OP.mult
    )

    # loss = (ncomp > 0) * tot / max(ncomp, 0.5)
    ncm = ep.tile([1, 1], F32, name="ncm")
    nc.vector.tensor_scalar(
        out=ncm[:], in0=ncomp[:], scalar1=0.5, scalar2=None, op0=OP.max
    )
    rnc = ep.tile([1, 1], F32, name="rnc")
    nc.vector.reciprocal(rnc[:], ncm[:])
    mask = ep.tile([1, 1], F32, name="mask")
    nc.vector.tensor_scalar(
        out=mask[:], in0=ncomp[:], scalar1=0.25, scalar2=None, op0=OP.is_ge
    )
    res = ep.tile([1, 1], F32, name="res")
    nc.vector.tensor_tensor(out=res[:], in0=tot[:], in1=rnc[:], op=OP.mult)
    nc.vector.tensor_tensor(out=res[:], in0=res[:], in1=mask[:], op=OP.mult)

    nc.sync.dma_start(loss_d[:], res[:])


def build(n_cores=NCORES):
    nc = bacc.Bacc(
        "TRN2", target_bir_lowering=False, debug=False, num_devices=n_cores
    )
    pred_d = nc.dram_tensor("pred", [ROWS_PAD, D], F32, kind="ExternalInput").ap()
    attr_d = nc.dram_tensor("attr", [ROWS_PAD, A], BF16, kind="ExternalInput").ap()
    loss_d = nc.dram_tensor("loss", [1, 1], F32, kind="ExternalOutput").ap()
    with tile.TileContext(nc) as tc:
        emit_kernel(tc, pred_d, attr_d, loss_d, n_cores=n_cores)
    nc.compile()
    return nc


def shard_inputs(predictions, attr_vals, n_cores=NCORES, rows_pad=ROWS_PAD):
    rows = predictions.shape[0] // n_cores
    in_maps = []
    for c in range(n_cores):
        p = predictions[c * rows : (c + 1) * rows]
        a = attr_vals[c * rows : (c + 1) * rows]
        pad = rows_pad - rows
        if pad:
            p = np.concatenate([p, np.zeros((pad, D), np.float32)], axis=0)
            a = np.concatenate([a, np.full((pad, A), PAD_ATTR, np.int32)], axis=0)
        a16 = a.astype(np.float32).astype(_BF16_NP)
        in_maps.append(
            {
                "pred": np.ascontiguousarray(p),
                "attr": np.ascontiguousarray(a16),
            }
        )
    return in_maps


try:
    import ml_dtypes

    _BF16_NP = ml_dtypes.bfloat16
except Exception:  # pragma: no cover
    import jax.numpy as jnp

    _BF16_NP = jnp.bfloat16

_NC_CACHE = {}


def kernel(predictions: np.ndarray, attr_vals: np.ndarray) -> np.ndarray:
    predictions = np.asarray(predictions, np.float32)
    attr_vals = np.asarray(attr_vals, np.int32)
    if "nc" not in _NC_CACHE:
        _NC_CACHE["nc"] = build()
    nc = _NC_CACHE["nc"]
    in_maps = shard_inputs(predictions, attr_vals)
    res = bass_utils.run_bass_kernel_spmd(nc, in_maps, list(range(NCORES)))
    return np.float32(res.results[0]["loss"][0, 0])
